# revision 1
# baseline (speedup 1.0000x reference)
"""Trainium2 Bass kernel for nn_CrossFeature (sparse_attention).

Math (per batch b):
    att[b,n,f]  = (x[b] @ W.T @ q.T).T * E**-0.5          # folded: x[b] @ (qW).T
    Xs          = 0.5 * att                               # entmax15 pre-scale
    gate        = entmax15(att) over f  (solved by Newton on the entmax root)
    out[b,n,e]  = exp( sum_f gate*value * x[b,f,e] )

Key algebraic moves:
  * stage-1/2 fused: qtilde = (q @ W) * 0.5 * E**-0.5, Xs = x @ qtilde.T
  * entmax15 bisection (50 iters) replaced by Newton on
        g(tau) = sum_f relu(Xs-tau)^2 - 1,
    with moments from bn_stats over m = max(Xs, tau):
        s1 = sum relu(Xs-tau)   = 32*((mean_e-tau)+(mean_o-tau))
        s2 = sum relu(Xs-tau)^2 = M2_e + M2_o + 32*((mean_e-tau)^2+(mean_o-tau)^2)
    init tau0 = mean - (cbar/2 + (1 - v64)/(128*cbar))  (linearized sqrt)
    3 Newton iterations reach fp32 roundoff (validated vs the reference).

Sharding: pure data-parallel, batch 2048 -> 8 cores x 256.
"""

import numpy as np

B_FULL, F, E, N = 2048, 64, 256, 64
NCORES = 8
B_LOC = B_FULL // NCORES

SCALE = 0.5 * (E ** -0.5)   # folds entmax's (alpha-1) into qtilde
CBAR = 0.097                # linearization point for sqrt((1-v64)/64)
NEWTON_ITERS = 3


def build_program(B_loc=B_LOC, NG=4):
    import concourse.tile as tile
    from concourse import bacc, mybir, masks

    f32 = mybir.dt.float32
    bf16 = mybir.dt.bfloat16
    Alu = mybir.AluOpType
    ACTF = mybir.ActivationFunctionType

    HALF = B_loc // 2
    C = HALF // NG            # batch-pairs per group
    assert C * NG == HALF and C % 4 == 0
    FSEG = 512 // F           # bn_stats segments per instruction (8)

    nc = bacc.Bacc("TRN2", debug=False, num_devices=NCORES)
    x_d = nc.dram_tensor("x", [B_loc, F, E], f32, kind="ExternalInput").ap()
    w_d = nc.dram_tensor("bilinear_w", [E, E], f32, kind="ExternalInput").ap()
    q_d = nc.dram_tensor("query", [N, E], f32, kind="ExternalInput").ap()
    v_d = nc.dram_tensor("value", [N, F], f32, kind="ExternalInput").ap()
    o_d = nc.dram_tensor("out", [B_loc, N, E], f32, kind="ExternalOutput").ap()

    K0 = 0.5 * CBAR + 1.0 / (128.0 * CBAR)
    KW = 1.0 / (128.0 * CBAR)

    with tile.TileContext(nc) as tc:
        with (
            tc.tile_pool(name="const", bufs=1) as constp,
            tc.tile_pool(name="xp", bufs=C + C // 2 + 2) as xpp,
            tc.tile_pool(name="xbf", bufs=6) as xbfp,
            tc.tile_pool(name="xt", bufs=10) as xtp,
            tc.tile_pool(name="xs", bufs=2) as xsp,
            tc.tile_pool(name="mb", bufs=2) as mbp,
            tc.tile_pool(name="aw", bufs=2) as awp,
            tc.tile_pool(name="st", bufs=2) as stp,
            tc.tile_pool(name="sm", bufs=3) as smp,
            tc.tile_pool(name="awt", bufs=3) as awtp,
            tc.tile_pool(name="osb", bufs=3) as osbp,
            tc.tile_pool(name="ps12", bufs=2, space="PSUM") as ps12p,
            tc.tile_pool(name="pst", bufs=2, space="PSUM") as pstp,
            tc.tile_pool(name="ps3", bufs=3, space="PSUM") as ps3p,
        ):
            # ---------------- constants ----------------
            ident = constp.tile([128, 128], f32)
            masks.make_identity(nc, ident[:])

            v2 = constp.tile([128, F], f32)
            nc.sync.dma_start(v2[0:64, :], v_d[:, :])
            nc.sync.dma_start(v2[64:128, :], v_d[:, :])

            wt = {}
            for di in range(2):
                for ej in range(2):
                    t = constp.tile([128, 128], f32, tag=f"wt{di}{ej}")
                    nc.sync.dma_start(
                        t[:], w_d[di * 128:(di + 1) * 128, ej * 128:(ej + 1) * 128]
                    )
                    wt[di, ej] = t

            qtin = []
            for di in range(2):
                t = constp.tile([128, N], f32, tag=f"qtin{di}")
                nc.sync.dma_start(
                    t[:], q_d[:, di * 128:(di + 1) * 128].transpose([1, 0])
                )
                qtin.append(t)

            # qtilde^T = W.T-contract: qt[e, n] = sum_d W[d, e] q[n, d], then * SCALE
            qt_bf = []
            for ej in range(2):
                ps = ps12p.tile([128, N], f32, tag="ps12")
                for di in range(2):
                    nc.tensor.matmul(
                        ps[:], wt[di, ej][:], qtin[di][:],
                        start=(di == 0), stop=(di == 1),
                    )
                t = constp.tile([128, N], bf16, tag=f"qtbf{ej}")
                nc.scalar.mul(t[:], ps[:], SCALE)
                qt_bf.append(t)

            # ---------------- per-group pipeline ----------------
            for g in range(NG):
                b0 = g * C                      # first half-1 batch of group
                # --- load x pairs, cast to bf16, transpose via xbar DMA ---
                xp_tiles = []
                xt_tiles = []
                for c in range(C):
                    bA = b0 + c
                    xp = xpp.tile([128, E], f32, tag="xp")
                    nc.sync.dma_start(xp[0:64, :], x_d[bA, :, :])
                    nc.sync.dma_start(xp[64:128, :], x_d[bA + HALF, :, :])
                    xp_tiles.append(xp)
                    xbf = xbfp.tile([128, E], bf16, tag="xbf")
                    nc.gpsimd.tensor_copy(xbf[:], xp[:])
                    pair_t = []
                    for ec in range(2):
                        xt = xtp.tile([128, 128], bf16, tag="xt")
                        nc.sync.dma_start_transpose(
                            xt[:], xbf[:, ec * 128:(ec + 1) * 128]
                        )
                        pair_t.append(xt)
                    xt_tiles.append(pair_t)

                # --- stage-12 matmuls: Xs[p=(n|n), c*64+f] ---
                xs_t = xsp.tile([128, C, 72], f32, tag="xs")
                xs3 = xs_t[:, :, 0:F]
                for blk in range(C // 8):
                    ps = ps12p.tile([128, 512], f32, tag="ps12")
                    for s in range(8):
                        c = blk * 8 + s
                        for ec in range(2):
                            nc.tensor.matmul(
                                ps[0:64, s * 64:(s + 1) * 64],
                                qt_bf[ec][:],
                                xt_tiles[c][ec][:, 0:64],
                                start=(ec == 0), stop=(ec == 1),
                                tile_position=(0, 0),
                                skip_group_check=True,
                            )
                        for ec in range(2):
                            nc.tensor.matmul(
                                ps[64:128, s * 64:(s + 1) * 64],
                                qt_bf[ec][:],
                                xt_tiles[c][ec][:, 64:128],
                                start=(ec == 0), stop=(ec == 1),
                                tile_position=(0, 64),
                                skip_group_check=True,
                            )
                    nc.scalar.copy(
                        xs3[:, blk * 8:(blk + 1) * 8, :],
                        ps[:].rearrange("p (c f) -> p c f", f=F),
                    )

                # --- entmax via Newton ---
                st = stp.tile([128, C, 8], f32, tag="st")

                def bn_pass(src3):
                    # HW BNStats: one segment per instruction (out = 6/partition)
                    for c in range(C):
                        nc.vector.bn_stats(
                            st[:, c, 0:6],
                            src3[:, c, :],
                        )

                def sl(k):
                    return st[:, :, k:k + 1]        # [128, C, 1]

                tau = smp.tile([128, C], f32, tag="tau")
                tauu = tau[:].unsqueeze(2)          # [128, C, 1]

                # init: tau0 = 0.5*msum + KW*wsum - K0
                bn_pass(xs3)
                msum = smp.tile([128, C], f32, tag="msum")
                wsum = smp.tile([128, C], f32, tag="wsum")
                nc.vector.tensor_add(msum[:].unsqueeze(2), sl(1), sl(4))
                nc.vector.tensor_add(wsum[:].unsqueeze(2), sl(2), sl(5))
                nc.vector.tensor_scalar(
                    out=msum[:], in0=msum[:], scalar1=0.5, scalar2=K0,
                    op0=Alu.mult, op1=Alu.subtract,
                )
                nc.vector.scalar_tensor_tensor(
                    out=tau[:], in0=wsum[:], scalar=KW, in1=msum[:],
                    op0=Alu.mult, op1=Alu.add,
                )

                mb_t = mbp.tile([128, C, 72], f32, tag="mb")
                mb3 = mb_t[:, :, 0:F]
                taub = tauu.broadcast_to([128, C, F])

                a2 = smp.tile([128, C, 2], f32, tag="a2")
                u2 = smp.tile([128, C, 2], f32, tag="u2")
                s1m = smp.tile([128, C], f32, tag="s1m")
                s2s = smp.tile([128, C], f32, tag="s2s")
                rcp = smp.tile([128, C], f32, tag="rcp")

                for it in range(NEWTON_ITERS + 1):
                    nc.vector.tensor_max(mb3, xs3, taub)
                    bn_pass(mb3)
                    # a = mean - tau (even, odd)
                    nc.vector.tensor_sub(
                        a2[:, :, 0:1], sl(1), tauu
                    )
                    nc.vector.tensor_sub(
                        a2[:, :, 1:2], sl(4), tauu
                    )
                    # sq = a*a on ACT; u = 32*sq + M2
                    nc.scalar.square(u2[:], a2[:])
                    nc.vector.scalar_tensor_tensor(
                        out=u2[:, :, 0:1], in0=u2[:, :, 0:1], scalar=32.0,
                        in1=sl(2), op0=Alu.mult, op1=Alu.add,
                    )
                    nc.vector.scalar_tensor_tensor(
                        out=u2[:, :, 1:2], in0=u2[:, :, 1:2], scalar=32.0,
                        in1=sl(5), op0=Alu.mult, op1=Alu.add,
                    )
                    nc.vector.tensor_reduce(
                        s2s[:], u2[:], axis=mybir.AxisListType.X, op=Alu.add,
                    )
                    if it < NEWTON_ITERS:
                        # dtau = (s2 - 1) / (64 * s1m);  s1m = ae + ao
                        nc.vector.tensor_reduce(
                            s1m[:], a2[:], axis=mybir.AxisListType.X, op=Alu.add,
                        )
                        nc.vector.reciprocal(rcp[:], s1m[:])
                        nc.vector.tensor_scalar(
                            out=s2s[:], in0=s2s[:], scalar1=-1.0, scalar2=None,
                            op0=Alu.add,
                        )
                        nc.vector.tensor_mul(s2s[:], s2s[:], rcp[:])
                        nc.vector.scalar_tensor_tensor(
                            out=tau[:], in0=s2s[:], scalar=1.0 / 64.0, in1=tau[:],
                            op0=Alu.mult, op1=Alu.add,
                        )

                # final: recip_s2, d = m - tau, aw = d^2 * v
                recs2 = smp.tile([128, C], f32, tag="recs2")
                nc.vector.reciprocal(recs2[:], s2s[:])
                nc.vector.tensor_sub(mb3, mb3, taub)
                aw_t = awp.tile([128, C * F], f32, tag="aw")
                aw3 = aw_t[:].rearrange("p (c f) -> p c f", f=F)
                nc.scalar.square(aw3, mb3)
                v2b = v2[:].unsqueeze(1).broadcast_to([128, C, F])
                nc.vector.tensor_mul(aw3, aw3, v2b)

                # --- stage-3: out[b] = exp( (aw_b)^T-weights @ x_b * 1/s2 ) ---
                for blk in range(C // 4):
                    # One full transpose per pair: S_c [128,64] -> [64,128] at
                    # PSUM partition 0 (HW requires transpose out/in base 0).
                    # Slot s holds [W1 | W2] at cols s*128 : s*128+128.
                    pst = pstp.tile([64, 512], f32, tag="pst")
                    for s in range(4):
                        c = blk * 4 + s
                        nc.tensor.transpose(
                            pst[0:64, s * 128:(s + 1) * 128],
                            aw_t[:, c * 64:(c + 1) * 64],
                            ident[:],
                        )
                    pst3 = pst[:].rearrange("p (s h f) -> p s h f", s=4, h=2)
                    awt = awtp.tile([128, 256], f32, tag="awt")
                    awt3 = awt[:].rearrange("p (s f) -> p s f", s=4)
                    nc.scalar.copy(awt3[0:64, :, :], pst3[:, :, 0, :])
                    # W2 must reach partitions 64:128: evac to SBUF first,
                    # then partition-shift with an SBUF->SBUF DMA.
                    awt_tmp = awtp.tile([64, 256], f32, tag="awt_tmp")
                    nc.scalar.copy(
                        awt_tmp[:].rearrange("p (s f) -> p s f", s=4),
                        pst3[:, :, 1, :],
                    )
                    nc.sync.dma_start(awt[64:128, :], awt_tmp[:])

                    osb = osbp.tile([128, 1024], f32, tag="osb")
                    for s in range(4):
                        c = blk * 4 + s
                        ps3 = ps3p.tile([128, E], f32, tag="ps3")
                        nc.tensor.matmul(
                            ps3[0:64, :],
                            awt[0:64, s * 64:(s + 1) * 64],
                            xp_tiles[c][0:64, :],
                            start=True, stop=True,
                            tile_position=(0, 0),
                            skip_group_check=True,
                        )
                        nc.tensor.matmul(
                            ps3[64:128, :],
                            awt[64:128, s * 64:(s + 1) * 64],
                            xp_tiles[c][64:128, :],
                            start=True, stop=True,
                            tile_position=(64, 64),
                            skip_group_check=True,
                        )
                        nc.scalar.activation(
                            osb[:, s * 256:(s + 1) * 256], ps3[:],
                            ACTF.Exp, scale=recs2[:, c:c + 1],
                        )
                    bA = b0 + blk * 4
                    nc.sync.dma_start(
                        o_d[bA:bA + 4, :, :].transpose([1, 0, 2]),
                        osb[0:64, :].rearrange("p (t e) -> p t e", t=4),
                    )
                    nc.sync.dma_start(
                        o_d[HALF + bA:HALF + bA + 4, :, :].transpose([1, 0, 2]),
                        osb[64:128, :].rearrange("p (t e) -> p t e", t=4),
                    )
    if not nc.is_finalized():
        nc.finalize()
    return nc


_NC_CACHE = {}


def _get_program(B_loc, NG):
    key = (B_loc, NG)
    if key not in _NC_CACHE:
        _NC_CACHE[key] = build_program(B_loc, NG)
    return _NC_CACHE[key]


def kernel(**inputs):
    from concourse.bass_utils import run_bass_kernel_spmd

    x = np.ascontiguousarray(np.asarray(inputs["x"], dtype=np.float32))
    w = np.ascontiguousarray(np.asarray(inputs["bilinear_w"], dtype=np.float32))
    q = np.ascontiguousarray(np.asarray(inputs["query"], dtype=np.float32))
    v = np.ascontiguousarray(np.asarray(inputs["value"], dtype=np.float32))
    B = x.shape[0]
    B_loc = B // NCORES

    nc = _get_program(B_loc, 4)

    in_maps = []
    for core in range(NCORES):
        sh = x[core * B_loc:(core + 1) * B_loc]
        in_maps.append(
            {"x": np.ascontiguousarray(sh), "bilinear_w": w, "query": q, "value": v}
        )

    import os
    trace = bool(int(os.environ.get("KERNEL_TRACE", "0")))
    res = run_bass_kernel_spmd(
        nc, in_maps, core_ids=list(range(NCORES)), trace=trace,
        trace_cores=[0] if trace else None,
    )
    if trace:
        kernel.last_exec_time_ns = res.exec_time_ns
        kernel.last_trace = res.instructions_and_trace
    out = np.concatenate([r["out"] for r in res.results], axis=0)
    return out


if __name__ == "__main__":
    # smoke-test the builder only
    nc = build_program(32, 2)
    print("build ok:", len(nc.inst_map), "instructions")



# revision 4
# speedup vs baseline: 3.2139x; 3.2139x over previous
"""Trainium2 Bass kernel for nn_CrossFeature (sparse_attention).

Math (per batch b):
    att[b,n,f]  = (x[b] @ W.T @ q.T).T * E**-0.5          # folded: x[b] @ (qW).T
    Xs          = 0.5 * att                               # entmax15 pre-scale
    gate        = entmax15(att) over f  (solved by Newton on the entmax root)
    out[b,n,e]  = exp( sum_f gate*value * x[b,f,e] )

Key algebraic moves:
  * stage-1/2 fused: qtilde = (q @ W) * 0.5 * E**-0.5, Xs = x @ qtilde.T
  * entmax15 bisection (50 iters) replaced by Newton on
        g(tau) = sum_f relu(Xs-tau)^2 - 1,
    with moments from bn_stats over m = max(Xs, tau):
        s1 = sum relu(Xs-tau)   = 32*((mean_e-tau)+(mean_o-tau))
        s2 = sum relu(Xs-tau)^2 = M2_e + M2_o + 32*((mean_e-tau)^2+(mean_o-tau)^2)
    init tau0 = mean - (cbar/2 + (1 - v64)/(128*cbar))  (linearized sqrt)
    3 Newton iterations reach fp32 roundoff (validated vs the reference).

Dataflow (v2):
  * x loaded in 8-pair batched DMAs into per-group tiles [128, C, E].
  * x^T tiles produced by PE-array transposes (fp32) into PSUM, evacuated
    with a fused fp32->bf16 cast on the scalar engine. (v1 used GpSimd
    casts + Sync-engine xbar DMA transposes, which serialized the whole
    kernel at ~6.3us/pair.)
  * stage-12 matmuls run e-chunk-outer so the qt weights stay resident
    per PE column-quadrant across 16 consecutive matmuls.
  * 1/s2 is folded into attn_weight before stage-3, so the final Exp
    activation is batched over 2 pairs (scale=1.0).

Sharding: pure data-parallel, batch 2048 -> 8 cores x 256.
"""

import numpy as np

B_FULL, F, E, N = 2048, 64, 256, 64
NCORES = 8
B_LOC = B_FULL // NCORES

SCALE = 0.5 * (E ** -0.5)   # folds entmax's (alpha-1) into qtilde
CBAR = 0.097                # linearization point for sqrt((1-v64)/64)
NEWTON_ITERS = 3


def build_program(B_loc=B_LOC, NG=4):
    import concourse.tile as tile
    from concourse import bacc, mybir, masks

    f32 = mybir.dt.float32
    bf16 = mybir.dt.bfloat16
    Alu = mybir.AluOpType
    ACTF = mybir.ActivationFunctionType

    HALF = B_loc // 2
    C = HALF // NG            # batch-pairs per group
    assert C * NG == HALF and C % 8 == 0

    nc = bacc.Bacc("TRN2", debug=False, num_devices=NCORES)
    x_d = nc.dram_tensor("x", [B_loc, F, E], f32, kind="ExternalInput").ap()
    w_d = nc.dram_tensor("bilinear_w", [E, E], f32, kind="ExternalInput").ap()
    q_d = nc.dram_tensor("query", [N, E], f32, kind="ExternalInput").ap()
    v_d = nc.dram_tensor("value", [N, F], f32, kind="ExternalInput").ap()
    o_d = nc.dram_tensor("out", [B_loc, N, E], f32, kind="ExternalOutput").ap()

    K0 = 0.5 * CBAR + 1.0 / (128.0 * CBAR)
    KW = 1.0 / (128.0 * CBAR)

    with tile.TileContext(nc) as tc:
        with (
            tc.tile_pool(name="const", bufs=1) as constp,
            tc.tile_pool(name="xpg", bufs=2) as xpgp,
            tc.tile_pool(name="xtg", bufs=2) as xtgp,
            tc.tile_pool(name="xs", bufs=2) as xsp,
            tc.tile_pool(name="mb", bufs=2) as mbp,
            tc.tile_pool(name="aw", bufs=2) as awp,
            tc.tile_pool(name="st", bufs=2) as stp,
            tc.tile_pool(name="sm", bufs=3) as smp,
            tc.tile_pool(name="awt", bufs=3) as awtp,
            tc.tile_pool(name="osb", bufs=3) as osbp,
            tc.tile_pool(name="pstx", bufs=2, space="PSUM") as pstxp,
            tc.tile_pool(name="ps12", bufs=2, space="PSUM") as ps12p,
            tc.tile_pool(name="ps3", bufs=3, space="PSUM") as ps3p,
            tc.tile_pool(name="psaw", bufs=1, space="PSUM") as psawp,
        ):
            # ---------------- constants ----------------
            ident = constp.tile([128, 128], f32)
            masks.make_identity(nc, ident[:])

            v2 = constp.tile([128, F], f32)
            nc.sync.dma_start(v2[0:64, :], v_d[:, :])
            nc.sync.dma_start(v2[64:128, :], v_d[:, :])

            wt = {}
            for di in range(2):
                for ej in range(2):
                    t = constp.tile([128, 128], f32, tag=f"wt{di}{ej}")
                    nc.sync.dma_start(
                        t[:], w_d[di * 128:(di + 1) * 128, ej * 128:(ej + 1) * 128]
                    )
                    wt[di, ej] = t

            qtin = []
            for di in range(2):
                t = constp.tile([128, N], f32, tag=f"qtin{di}")
                nc.sync.dma_start(
                    t[:], q_d[:, di * 128:(di + 1) * 128].transpose([1, 0])
                )
                qtin.append(t)

            # qtilde^T = W.T-contract: qt[e, n] = sum_d W[d, e] q[n, d], then * SCALE
            qt_bf = []
            for ej in range(2):
                ps = ps12p.tile([128, 512], f32, tag="ps12")
                for di in range(2):
                    nc.tensor.matmul(
                        ps[:, 0:N], wt[di, ej][:], qtin[di][:],
                        start=(di == 0), stop=(di == 1),
                    )
                t = constp.tile([128, N], bf16, tag=f"qtbf{ej}")
                nc.scalar.mul(t[:], ps[:, 0:N], SCALE)
                qt_bf.append(t)

            # ---------------- per-group pipeline ----------------
            for g in range(NG):
                b0 = g * C                      # first half-A batch of group
                # --- batched x loads: 8 pairs per DMA ---
                xpg = xpgp.tile([128, C, E], f32, tag="xpg")
                for q in range(C // 8):
                    s0 = q * 8
                    nc.sync.dma_start(
                        xpg[0:64, s0:s0 + 8, :],
                        x_d[b0 + s0:b0 + s0 + 8, :, :].transpose([1, 0, 2]),
                    )
                    nc.sync.dma_start(
                        xpg[64:128, s0:s0 + 8, :],
                        x_d[HALF + b0 + s0:HALF + b0 + s0 + 8, :, :]
                        .transpose([1, 0, 2]),
                    )

                # --- x^T via PE transpose, 2 pairs per PSUM bank, cast evac ---
                xtg = xtgp.tile([128, C, 2, 128], bf16, tag="xtg")
                for c2 in range(C // 2):
                    pst = pstxp.tile([128, 2, 2, 128], f32, tag="pstx")
                    for k in range(2):
                        c = c2 * 2 + k
                        for ec in range(2):
                            nc.tensor.transpose(
                                pst[:, k, ec, :],
                                xpg[:, c, ec * 128:(ec + 1) * 128],
                                ident[:],
                            )
                    nc.scalar.copy(xtg[:, c2 * 2:c2 * 2 + 2, :, :], pst[:])

                # --- stage-12 matmuls: Xs[p=(n|n), c, f], e-chunk outer ---
                xs_t = xsp.tile([128, C, 72], f32, tag="xs")
                xs3 = xs_t[:, :, 0:F]
                for blk in range(C // 8):
                    ps = ps12p.tile([128, 512], f32, tag="ps12")
                    for ec in range(2):
                        for s in range(8):
                            c = blk * 8 + s
                            # start=True clears has_written for the MM's
                            # partition rows across the WHOLE bank — so only
                            # the first MM of each row-half may set it, or
                            # later accumulates get overwritten.
                            nc.tensor.matmul(
                                ps[0:64, s * 64:(s + 1) * 64],
                                qt_bf[ec][:],
                                xtg[:, c, ec, 0:64],
                                start=(ec == 0 and s == 0), stop=(ec == 1),
                                tile_position=(0, 0),
                                skip_group_check=True,
                            )
                            nc.tensor.matmul(
                                ps[64:128, s * 64:(s + 1) * 64],
                                qt_bf[ec][:],
                                xtg[:, c, ec, 64:128],
                                start=(ec == 0 and s == 0), stop=(ec == 1),
                                tile_position=(0, 64),
                                skip_group_check=True,
                            )
                    nc.vector.tensor_copy(
                        xs3[:, blk * 8:(blk + 1) * 8, :],
                        ps[:].rearrange("p (c f) -> p c f", f=F),
                    )

                # --- entmax via Newton ---
                st = stp.tile([128, C, 8], f32, tag="st")

                def bn_pass(src3):
                    # HW BNStats: one segment per instruction (out = 6/partition)
                    for c in range(C):
                        nc.vector.bn_stats(
                            st[:, c, 0:6],
                            src3[:, c, :],
                        )

                def sl(k):
                    return st[:, :, k:k + 1]        # [128, C, 1]

                tau = smp.tile([128, C], f32, tag="tau")
                tauu = tau[:].unsqueeze(2)          # [128, C, 1]

                # init: tau0 = 0.5*msum + KW*wsum - K0
                bn_pass(xs3)
                msum = smp.tile([128, C], f32, tag="msum")
                wsum = smp.tile([128, C], f32, tag="wsum")
                nc.vector.tensor_add(msum[:].unsqueeze(2), sl(1), sl(4))
                nc.vector.tensor_add(wsum[:].unsqueeze(2), sl(2), sl(5))
                nc.vector.tensor_scalar(
                    out=msum[:], in0=msum[:], scalar1=0.5, scalar2=K0,
                    op0=Alu.mult, op1=Alu.subtract,
                )
                nc.vector.scalar_tensor_tensor(
                    out=tau[:], in0=wsum[:], scalar=KW, in1=msum[:],
                    op0=Alu.mult, op1=Alu.add,
                )

                mb_t = mbp.tile([128, C, 72], f32, tag="mb")
                mb3 = mb_t[:, :, 0:F]
                taub = tauu.broadcast_to([128, C, F])

                a2 = smp.tile([128, C, 2], f32, tag="a2")
                u2 = smp.tile([128, C, 2], f32, tag="u2")
                s1m = smp.tile([128, C], f32, tag="s1m")
                s2s = smp.tile([128, C], f32, tag="s2s")
                rcp = smp.tile([128, C], f32, tag="rcp")

                for it in range(NEWTON_ITERS + 1):
                    nc.vector.tensor_max(mb3, xs3, taub)
                    bn_pass(mb3)
                    # a = mean - tau (even, odd)
                    nc.vector.tensor_sub(
                        a2[:, :, 0:1], sl(1), tauu
                    )
                    nc.vector.tensor_sub(
                        a2[:, :, 1:2], sl(4), tauu
                    )
                    # sq = a*a on ACT; u = 32*sq + M2
                    nc.scalar.square(u2[:], a2[:])
                    nc.vector.scalar_tensor_tensor(
                        out=u2[:, :, 0:1], in0=u2[:, :, 0:1], scalar=32.0,
                        in1=sl(2), op0=Alu.mult, op1=Alu.add,
                    )
                    nc.vector.scalar_tensor_tensor(
                        out=u2[:, :, 1:2], in0=u2[:, :, 1:2], scalar=32.0,
                        in1=sl(5), op0=Alu.mult, op1=Alu.add,
                    )
                    nc.vector.tensor_reduce(
                        s2s[:], u2[:], axis=mybir.AxisListType.X, op=Alu.add,
                    )
                    if it < NEWTON_ITERS:
                        # dtau = (s2 - 1) / (64 * s1m);  s1m = ae + ao
                        nc.vector.tensor_reduce(
                            s1m[:], a2[:], axis=mybir.AxisListType.X, op=Alu.add,
                        )
                        nc.vector.reciprocal(rcp[:], s1m[:])
                        nc.vector.tensor_scalar(
                            out=s2s[:], in0=s2s[:], scalar1=-1.0, scalar2=None,
                            op0=Alu.add,
                        )
                        nc.vector.tensor_mul(s2s[:], s2s[:], rcp[:])
                        nc.vector.scalar_tensor_tensor(
                            out=tau[:], in0=s2s[:], scalar=1.0 / 64.0, in1=tau[:],
                            op0=Alu.mult, op1=Alu.add,
                        )

                # final: recip_s2, d = m - tau, aw = d^2 * v * recip_s2
                recs2 = smp.tile([128, C], f32, tag="recs2")
                nc.vector.reciprocal(recs2[:], s2s[:])
                nc.vector.tensor_sub(mb3, mb3, taub)
                aw_t = awp.tile([128, C * F], f32, tag="aw")
                aw3 = aw_t[:].rearrange("p (c f) -> p c f", f=F)
                nc.scalar.square(aw3, mb3)
                v2b = v2[:].unsqueeze(1).broadcast_to([128, C, F])
                nc.vector.tensor_mul(aw3, aw3, v2b)
                recs2b = recs2[:].unsqueeze(2).broadcast_to([128, C, F])
                nc.vector.tensor_mul(aw3, aw3, recs2b)

                # --- stage-3: out[b] = exp( (aw_b)^T-weights @ x_b ) ---
                for blk in range(C // 4):
                    # One full transpose per pair: S_c [128,64] -> [64,128] at
                    # PSUM partition 0 (HW requires transpose out/in base 0).
                    # Slot s holds [W1 | W2] at cols s*128 : s*128+128.
                    pst = psawp.tile([64, 512], f32, tag="psaw")
                    for s in range(4):
                        c = blk * 4 + s
                        nc.tensor.transpose(
                            pst[0:64, s * 128:(s + 1) * 128],
                            aw_t[:, c * 64:(c + 1) * 64],
                            ident[:],
                        )
                    pst3 = pst[:].rearrange("p (s h f) -> p s h f", s=4, h=2)
                    awt = awtp.tile([128, 256], f32, tag="awt")
                    awt3 = awt[:].rearrange("p (s f) -> p s f", s=4)
                    nc.scalar.copy(awt3[0:64, :, :], pst3[:, :, 0, :])
                    # W2 must reach partitions 64:128: evac to SBUF first,
                    # then partition-shift with an SBUF->SBUF DMA.
                    awt_tmp = awtp.tile([64, 256], f32, tag="awt_tmp")
                    nc.scalar.copy(
                        awt_tmp[:].rearrange("p (s f) -> p s f", s=4),
                        pst3[:, :, 1, :],
                    )
                    nc.sync.dma_start(awt[64:128, :], awt_tmp[:])

                    osb = osbp.tile([128, 1024], f32, tag="osb")
                    for s2p in range(2):
                        ps3 = ps3p.tile([128, 512], f32, tag="ps3")
                        for sl2 in range(2):
                            s = s2p * 2 + sl2
                            c = blk * 4 + s
                            nc.tensor.matmul(
                                ps3[0:64, sl2 * 256:(sl2 + 1) * 256],
                                awt[0:64, s * 64:(s + 1) * 64],
                                xpg[0:64, c, :],
                                start=True, stop=True,
                                tile_position=(0, 0),
                                skip_group_check=True,
                            )
                            nc.tensor.matmul(
                                ps3[64:128, sl2 * 256:(sl2 + 1) * 256],
                                awt[64:128, s * 64:(s + 1) * 64],
                                xpg[64:128, c, :],
                                start=True, stop=True,
                                tile_position=(64, 64),
                                skip_group_check=True,
                            )
                        nc.scalar.activation(
                            osb[:, s2p * 512:(s2p + 1) * 512], ps3[:],
                            ACTF.Exp,
                        )
                    bA = b0 + blk * 4
                    nc.sync.dma_start(
                        o_d[bA:bA + 4, :, :].transpose([1, 0, 2]),
                        osb[0:64, :].rearrange("p (t e) -> p t e", t=4),
                    )
                    nc.sync.dma_start(
                        o_d[HALF + bA:HALF + bA + 4, :, :].transpose([1, 0, 2]),
                        osb[64:128, :].rearrange("p (t e) -> p t e", t=4),
                    )
    if not nc.is_finalized():
        nc.finalize()
    return nc


_NC_CACHE = {}


def _get_program(B_loc, NG):
    key = (B_loc, NG)
    if key not in _NC_CACHE:
        _NC_CACHE[key] = build_program(B_loc, NG)
    return _NC_CACHE[key]


def kernel(**inputs):
    from concourse.bass_utils import run_bass_kernel_spmd

    x = np.ascontiguousarray(np.asarray(inputs["x"], dtype=np.float32))
    w = np.ascontiguousarray(np.asarray(inputs["bilinear_w"], dtype=np.float32))
    q = np.ascontiguousarray(np.asarray(inputs["query"], dtype=np.float32))
    v = np.ascontiguousarray(np.asarray(inputs["value"], dtype=np.float32))
    B = x.shape[0]
    B_loc = B // NCORES

    nc = _get_program(B_loc, 4)

    in_maps = []
    for core in range(NCORES):
        sh = x[core * B_loc:(core + 1) * B_loc]
        in_maps.append(
            {"x": np.ascontiguousarray(sh), "bilinear_w": w, "query": q, "value": v}
        )

    import os
    trace = bool(int(os.environ.get("KERNEL_TRACE", "0")))
    res = run_bass_kernel_spmd(
        nc, in_maps, core_ids=list(range(NCORES)), trace=trace,
        trace_cores=[0] if trace else None,
    )
    if trace:
        kernel.last_exec_time_ns = res.exec_time_ns
        kernel.last_trace = res.instructions_and_trace
    out = np.concatenate([r["out"] for r in res.results], axis=0)
    return out


if __name__ == "__main__":
    # smoke-test the builder only
    nc = build_program(32, 2)
    print("build ok:", len(nc.inst_map), "instructions")


# revision 5
# speedup vs baseline: 3.4472x; 1.0726x over previous
"""Trainium2 Bass kernel for nn_CrossFeature (sparse_attention).

Math (per batch b):
    att[b,n,f]  = (x[b] @ W.T @ q.T).T * E**-0.5          # folded: x[b] @ (qW).T
    Xs          = 0.5 * att                               # entmax15 pre-scale
    gate        = entmax15(att) over f  (solved by Newton on the entmax root)
    out[b,n,e]  = exp( sum_f gate*value * x[b,f,e] )

Key algebraic moves:
  * stage-1/2 fused: qtilde = (q @ W) * 0.5 * E**-0.5, Xs = x @ qtilde.T
  * entmax15 bisection (50 iters) replaced by Newton on
        g(tau) = sum_f relu(Xs-tau)^2 - 1,
    with moments from bn_stats over m = max(Xs, tau):
        s1 = sum relu(Xs-tau)   = 32*((mean_e-tau)+(mean_o-tau))
        s2 = sum relu(Xs-tau)^2 = M2_e + M2_o + 32*((mean_e-tau)^2+(mean_o-tau)^2)
    init tau0 = mean - (cbar/2 + (1 - v64)/(128*cbar))  (linearized sqrt)

Dataflow (v3):
  * x loaded ONCE, cast fp32->bf16 during the DMA itself (SWDGE on GpSimd),
    in 8-pair batched transfers.
  * x^T tiles produced by PE-array transposes (bf16) into PSUM, evacuated
    on the scalar engine. No xbar DMA transposes, no GpSimd compute casts.
  * stage-12 matmuls run e-chunk-outer with one start=True per PSUM
    row-half (start clears has_written for the MM's rows across the whole
    bank), so qt weights stay quadrant-resident for 16 matmuls.
  * stage-3 runs fully in bf16 (4x the fp32 PE throughput).
  * 1/s2 folded into attn_weight; Exp batched over 2 pairs.
  * output staged per half-group -> 2 big store DMAs per half-group.

Sharding: pure data-parallel, batch 2048 -> 8 cores x 256.
"""

import numpy as np

B_FULL, F, E, N = 2048, 64, 256, 64
NCORES = 8
B_LOC = B_FULL // NCORES

SCALE = 0.5 * (E ** -0.5)   # folds entmax's (alpha-1) into qtilde
CBAR = 0.097                # linearization point for sqrt((1-v64)/64)
NEWTON_ITERS = 2


def build_program(B_loc=B_LOC, NG=4):
    import concourse.tile as tile
    from concourse import bacc, mybir, masks

    f32 = mybir.dt.float32
    bf16 = mybir.dt.bfloat16
    Alu = mybir.AluOpType
    ACTF = mybir.ActivationFunctionType

    HALF = B_loc // 2
    C = HALF // NG            # batch-pairs per group
    assert C * NG == HALF and C % 8 == 0

    nc = bacc.Bacc("TRN2", debug=False, num_devices=NCORES)
    x_d = nc.dram_tensor("x", [B_loc, F, E], f32, kind="ExternalInput").ap()
    w_d = nc.dram_tensor("bilinear_w", [E, E], f32, kind="ExternalInput").ap()
    q_d = nc.dram_tensor("query", [N, E], f32, kind="ExternalInput").ap()
    v_d = nc.dram_tensor("value", [N, F], f32, kind="ExternalInput").ap()
    o_d = nc.dram_tensor("out", [B_loc, N, E], f32, kind="ExternalOutput").ap()

    K0 = 0.5 * CBAR + 1.0 / (128.0 * CBAR)
    KW = 1.0 / (128.0 * CBAR)

    with tile.TileContext(nc) as tc:
        with (
            tc.tile_pool(name="const", bufs=1) as constp,
            tc.tile_pool(name="xbf", bufs=2) as xbfp,
            tc.tile_pool(name="xtg", bufs=2) as xtgp,
            tc.tile_pool(name="xs", bufs=2) as xsp,
            tc.tile_pool(name="mb", bufs=2) as mbp,
            tc.tile_pool(name="aw", bufs=2) as awp,
            tc.tile_pool(name="st", bufs=2) as stp,
            tc.tile_pool(name="sm", bufs=3) as smp,
            tc.tile_pool(name="awt", bufs=2) as awtp,
            tc.tile_pool(name="osb", bufs=2) as osbp,
            tc.tile_pool(name="pstx", bufs=2, space="PSUM") as pstxp,
            tc.tile_pool(name="ps12", bufs=2, space="PSUM") as ps12p,
            tc.tile_pool(name="ps3", bufs=3, space="PSUM") as ps3p,
            tc.tile_pool(name="psaw", bufs=1, space="PSUM") as psawp,
        ):
            # ---------------- constants ----------------
            ident = constp.tile([128, 128], f32)
            masks.make_identity(nc, ident[:])
            ident_bf = constp.tile([128, 128], bf16, tag="identbf")
            nc.gpsimd.tensor_copy(ident_bf[:], ident[:])

            v2 = constp.tile([128, F], f32)
            nc.sync.dma_start(v2[0:64, :], v_d[:, :])
            nc.sync.dma_start(v2[64:128, :], v_d[:, :])

            wt = {}
            for di in range(2):
                for ej in range(2):
                    t = constp.tile([128, 128], f32, tag=f"wt{di}{ej}")
                    nc.sync.dma_start(
                        t[:], w_d[di * 128:(di + 1) * 128, ej * 128:(ej + 1) * 128]
                    )
                    wt[di, ej] = t

            qtin = []
            for di in range(2):
                t = constp.tile([128, N], f32, tag=f"qtin{di}")
                nc.sync.dma_start(
                    t[:], q_d[:, di * 128:(di + 1) * 128].transpose([1, 0])
                )
                qtin.append(t)

            # qtilde^T = W.T-contract: qt[e, n] = sum_d W[d, e] q[n, d], then * SCALE
            qt_bf = []
            for ej in range(2):
                ps = ps12p.tile([128, 512], f32, tag="ps12")
                for di in range(2):
                    nc.tensor.matmul(
                        ps[:, 0:N], wt[di, ej][:], qtin[di][:],
                        start=(di == 0), stop=(di == 1),
                    )
                t = constp.tile([128, N], bf16, tag=f"qtbf{ej}")
                nc.scalar.mul(t[:], ps[:, 0:N], SCALE)
                qt_bf.append(t)

            # ---------------- per-group pipeline ----------------
            for g in range(NG):
                b0 = g * C                      # first half-A batch of group
                # --- batched x loads with fp32->bf16 cast in the DMA ---
                xbf = xbfp.tile([128, C, E], bf16, tag="xbf")
                for q in range(C // 8):
                    s0 = q * 8
                    nc.gpsimd.dma_start(
                        xbf[0:64, s0:s0 + 8, :],
                        x_d[b0 + s0:b0 + s0 + 8, :, :].transpose([1, 0, 2]),
                    )
                    nc.gpsimd.dma_start(
                        xbf[64:128, s0:s0 + 8, :],
                        x_d[HALF + b0 + s0:HALF + b0 + s0 + 8, :, :]
                        .transpose([1, 0, 2]),
                    )

                # --- x^T via PE transpose (bf16), 2 pairs per PSUM tile ---
                xtg = xtgp.tile([128, C, 2, 128], bf16, tag="xtg")
                for c2 in range(C // 2):
                    pst = pstxp.tile([128, 2, 2, 128], bf16, tag="pstx")
                    for k in range(2):
                        c = c2 * 2 + k
                        for ec in range(2):
                            nc.tensor.transpose(
                                pst[:, k, ec, :],
                                xbf[:, c, ec * 128:(ec + 1) * 128],
                                ident_bf[:],
                            )
                    nc.scalar.copy(xtg[:, c2 * 2:c2 * 2 + 2, :, :], pst[:])

                # --- stage-12 matmuls: Xs[p=(n|n), c, f], e-chunk outer ---
                xs_t = xsp.tile([128, C, 72], f32, tag="xs")
                xs3 = xs_t[:, :, 0:F]
                for blk in range(C // 8):
                    ps = ps12p.tile([128, 512], f32, tag="ps12")
                    for ec in range(2):
                        for s in range(8):
                            c = blk * 8 + s
                            # start=True clears has_written for the MM's
                            # partition rows across the WHOLE bank — only
                            # the first MM of each row-half may set it.
                            nc.tensor.matmul(
                                ps[0:64, s * 64:(s + 1) * 64],
                                qt_bf[ec][:],
                                xtg[:, c, ec, 0:64],
                                start=(ec == 0 and s == 0), stop=(ec == 1),
                                tile_position=(0, 0),
                                skip_group_check=True,
                            )
                            nc.tensor.matmul(
                                ps[64:128, s * 64:(s + 1) * 64],
                                qt_bf[ec][:],
                                xtg[:, c, ec, 64:128],
                                start=(ec == 0 and s == 0), stop=(ec == 1),
                                tile_position=(0, 64),
                                skip_group_check=True,
                            )
                    nc.vector.tensor_copy(
                        xs3[:, blk * 8:(blk + 1) * 8, :],
                        ps[:].rearrange("p (c f) -> p c f", f=F),
                    )

                # --- entmax via Newton ---
                st = stp.tile([128, C, 8], f32, tag="st")

                def bn_pass(src3):
                    # HW BNStats: one segment per instruction (out = 6/partition)
                    for c in range(C):
                        nc.vector.bn_stats(
                            st[:, c, 0:6],
                            src3[:, c, :],
                        )

                def sl(k):
                    return st[:, :, k:k + 1]        # [128, C, 1]

                tau = smp.tile([128, C], f32, tag="tau")
                tauu = tau[:].unsqueeze(2)          # [128, C, 1]

                # init: tau0 = 0.5*msum + KW*wsum - K0
                bn_pass(xs3)
                msum = smp.tile([128, C], f32, tag="msum")
                wsum = smp.tile([128, C], f32, tag="wsum")
                nc.vector.tensor_add(msum[:].unsqueeze(2), sl(1), sl(4))
                nc.vector.tensor_add(wsum[:].unsqueeze(2), sl(2), sl(5))
                nc.vector.tensor_scalar(
                    out=msum[:], in0=msum[:], scalar1=0.5, scalar2=K0,
                    op0=Alu.mult, op1=Alu.subtract,
                )
                nc.vector.scalar_tensor_tensor(
                    out=tau[:], in0=wsum[:], scalar=KW, in1=msum[:],
                    op0=Alu.mult, op1=Alu.add,
                )

                mb_t = mbp.tile([128, C, 72], f32, tag="mb")
                mb3 = mb_t[:, :, 0:F]
                taub = tauu.broadcast_to([128, C, F])

                a2 = smp.tile([128, C, 2], f32, tag="a2")
                u2 = smp.tile([128, C, 2], f32, tag="u2")
                s1m = smp.tile([128, C], f32, tag="s1m")
                s2s = smp.tile([128, C], f32, tag="s2s")
                rcp = smp.tile([128, C], f32, tag="rcp")

                for it in range(NEWTON_ITERS + 1):
                    nc.vector.tensor_max(mb3, xs3, taub)
                    bn_pass(mb3)
                    # a = mean - tau (even, odd)
                    nc.vector.tensor_sub(
                        a2[:, :, 0:1], sl(1), tauu
                    )
                    nc.vector.tensor_sub(
                        a2[:, :, 1:2], sl(4), tauu
                    )
                    # sq = a*a on ACT; u = 32*sq + M2
                    nc.scalar.square(u2[:], a2[:])
                    nc.vector.scalar_tensor_tensor(
                        out=u2[:, :, 0:1], in0=u2[:, :, 0:1], scalar=32.0,
                        in1=sl(2), op0=Alu.mult, op1=Alu.add,
                    )
                    nc.vector.scalar_tensor_tensor(
                        out=u2[:, :, 1:2], in0=u2[:, :, 1:2], scalar=32.0,
                        in1=sl(5), op0=Alu.mult, op1=Alu.add,
                    )
                    nc.vector.tensor_reduce(
                        s2s[:], u2[:], axis=mybir.AxisListType.X, op=Alu.add,
                    )
                    if it < NEWTON_ITERS:
                        # dtau = (s2 - 1) / (64 * s1m);  s1m = ae + ao
                        nc.vector.tensor_reduce(
                            s1m[:], a2[:], axis=mybir.AxisListType.X, op=Alu.add,
                        )
                        nc.vector.reciprocal(rcp[:], s1m[:])
                        nc.vector.tensor_scalar(
                            out=s2s[:], in0=s2s[:], scalar1=-1.0, scalar2=None,
                            op0=Alu.add,
                        )
                        nc.vector.tensor_mul(s2s[:], s2s[:], rcp[:])
                        nc.vector.scalar_tensor_tensor(
                            out=tau[:], in0=s2s[:], scalar=1.0 / 64.0, in1=tau[:],
                            op0=Alu.mult, op1=Alu.add,
                        )

                # final: recip_s2, d = m - tau, aw = d^2 * v * recip_s2
                recs2 = smp.tile([128, C], f32, tag="recs2")
                nc.vector.reciprocal(recs2[:], s2s[:])
                nc.vector.tensor_sub(mb3, mb3, taub)
                aw_t = awp.tile([128, C * F], f32, tag="aw")
                aw3 = aw_t[:].rearrange("p (c f) -> p c f", f=F)
                nc.scalar.square(aw3, mb3)
                v2b = v2[:].unsqueeze(1).broadcast_to([128, C, F])
                nc.vector.tensor_mul(aw3, aw3, v2b)
                recs2b = recs2[:].unsqueeze(2).broadcast_to([128, C, F])
                nc.vector.tensor_mul(aw3, aw3, recs2b)

                # --- aw^T: PE transposes + cast evac; one shift DMA/group ---
                awt_g = awtp.tile([128, C, 64], bf16, tag="awt")
                awt_tmp = awtp.tile([64, C, 64], bf16, tag="awt_tmp")
                for blk in range(C // 4):
                    pst = psawp.tile([64, 512], f32, tag="psaw")
                    for s in range(4):
                        c = blk * 4 + s
                        nc.tensor.transpose(
                            pst[0:64, s * 128:(s + 1) * 128],
                            aw_t[:, c * 64:(c + 1) * 64],
                            ident[:],
                        )
                    pst3 = pst[:].rearrange("p (s h f) -> p s h f", s=4, h=2)
                    nc.scalar.copy(
                        awt_g[0:64, blk * 4:(blk + 1) * 4, :], pst3[:, :, 0, :]
                    )
                    nc.scalar.copy(
                        awt_tmp[:, blk * 4:(blk + 1) * 4, :], pst3[:, :, 1, :]
                    )
                # W2 -> partitions 64:128 via one SBUF->SBUF partition shift
                nc.sync.dma_start(awt_g[64:128, :, :], awt_tmp[:])

                # --- stage-3 (bf16): out = exp(awt.T @ x), staged per half ---
                for hb in range(2):
                    osb = osbp.tile([128, C // 2, E], f32, tag="osb")
                    for p2 in range(C // 4):
                        ps3 = ps3p.tile([128, 512], f32, tag="ps3")
                        for sl2 in range(2):
                            c = hb * (C // 2) + p2 * 2 + sl2
                            nc.tensor.matmul(
                                ps3[0:64, sl2 * 256:(sl2 + 1) * 256],
                                awt_g[0:64, c, :],
                                xbf[0:64, c, :],
                                start=True, stop=True,
                                tile_position=(0, 0),
                                skip_group_check=True,
                            )
                            nc.tensor.matmul(
                                ps3[64:128, sl2 * 256:(sl2 + 1) * 256],
                                awt_g[64:128, c, :],
                                xbf[64:128, c, :],
                                start=True, stop=True,
                                tile_position=(64, 64),
                                skip_group_check=True,
                            )
                        nc.scalar.activation(
                            osb[:, p2 * 2:p2 * 2 + 2, :], ps3[:],
                            ACTF.Exp,
                        )
                    bA = b0 + hb * (C // 2)
                    nc.sync.dma_start(
                        o_d[bA:bA + C // 2, :, :].transpose([1, 0, 2]),
                        osb[0:64, :, :],
                    )
                    nc.sync.dma_start(
                        o_d[HALF + bA:HALF + bA + C // 2, :, :]
                        .transpose([1, 0, 2]),
                        osb[64:128, :, :],
                    )
    if not nc.is_finalized():
        nc.finalize()
    return nc


_NC_CACHE = {}


def _get_program(B_loc, NG):
    key = (B_loc, NG)
    if key not in _NC_CACHE:
        _NC_CACHE[key] = build_program(B_loc, NG)
    return _NC_CACHE[key]


def kernel(**inputs):
    from concourse.bass_utils import run_bass_kernel_spmd

    x = np.ascontiguousarray(np.asarray(inputs["x"], dtype=np.float32))
    w = np.ascontiguousarray(np.asarray(inputs["bilinear_w"], dtype=np.float32))
    q = np.ascontiguousarray(np.asarray(inputs["query"], dtype=np.float32))
    v = np.ascontiguousarray(np.asarray(inputs["value"], dtype=np.float32))
    B = x.shape[0]
    B_loc = B // NCORES

    nc = _get_program(B_loc, 4)

    in_maps = []
    for core in range(NCORES):
        sh = x[core * B_loc:(core + 1) * B_loc]
        in_maps.append(
            {"x": np.ascontiguousarray(sh), "bilinear_w": w, "query": q, "value": v}
        )

    import os
    trace = bool(int(os.environ.get("KERNEL_TRACE", "0")))
    res = run_bass_kernel_spmd(
        nc, in_maps, core_ids=list(range(NCORES)), trace=trace,
        trace_cores=[0] if trace else None,
    )
    if trace:
        kernel.last_exec_time_ns = res.exec_time_ns
        kernel.last_trace = res.instructions_and_trace
    out = np.concatenate([r["out"] for r in res.results], axis=0)
    return out


if __name__ == "__main__":
    # smoke-test the builder only
    nc = build_program(32, 2)
    print("build ok:", len(nc.inst_map), "instructions")


# revision 6
# speedup vs baseline: 3.8038x; 1.1035x over previous
"""Trainium2 Bass kernel for nn_CrossFeature (sparse_attention).

Math (per batch b):
    att[b,n,f]  = (x[b] @ W.T @ q.T).T * E**-0.5          # folded: x[b] @ (qW).T
    Xs          = 0.5 * att                               # entmax15 pre-scale
    gate        = entmax15(att) over f  (solved by Newton on the entmax root)
    out[b,n,e]  = exp( sum_f gate*value * x[b,f,e] )

Key algebraic moves:
  * stage-1/2 fused: qtilde = (q @ W) * 0.5 * E**-0.5, Xs = x @ qtilde.T
  * entmax15 bisection (50 iters) replaced by Newton on
        g(tau) = sum_f relu(Xs-tau)^2 - 1,
    with moments from bn_stats over m = max(Xs, tau):
        s1 = sum relu(Xs-tau)   = 32*((mean_e-tau)+(mean_o-tau))
        s2 = sum relu(Xs-tau)^2 = M2_e + M2_o + 32*((mean_e-tau)^2+(mean_o-tau)^2)
    init tau0 = mean - (cbar/2 + (1 - v64)/(128*cbar))  (linearized sqrt)

Dataflow (v4):
  * x loaded ONCE, cast fp32->bf16 inside the DMA (SWDGE on GpSimd).
  * x^T via PE-array transposes (bf16) -> PSUM -> scalar-engine evac.
  * stage-12 e-chunk-outer with one start=True per PSUM row-half
    (start clears has_written for the MM's rows across the whole bank).
  * stage-3 fully bf16.  1/s2 folded into attn_weight; Exp batched x2.
  * groups processed TWO AT A TIME with their entmax instruction streams
    interleaved op-by-op: the vector engine is the pacing engine, and
    interleaving fills each group's Newton dependency gaps with the
    sibling group's work (per-engine queues are strict FIFO).

Sharding: pure data-parallel, batch 2048 -> 8 cores x 256.
"""

import numpy as np

B_FULL, F, E, N = 2048, 64, 256, 64
NCORES = 8
B_LOC = B_FULL // NCORES

SCALE = 0.5 * (E ** -0.5)   # folds entmax's (alpha-1) into qtilde
CBAR = 0.097                # linearization point for sqrt((1-v64)/64)
NEWTON_ITERS = 2


def build_program(B_loc=B_LOC, NG=8):
    import concourse.tile as tile
    from concourse import bacc, mybir, masks

    f32 = mybir.dt.float32
    bf16 = mybir.dt.bfloat16
    Alu = mybir.AluOpType
    ACTF = mybir.ActivationFunctionType

    HALF = B_loc // 2
    C = HALF // NG            # batch-pairs per group
    assert C * NG == HALF and C % 8 == 0 and NG % 2 == 0

    nc = bacc.Bacc("TRN2", debug=False, num_devices=NCORES)
    x_d = nc.dram_tensor("x", [B_loc, F, E], f32, kind="ExternalInput").ap()
    w_d = nc.dram_tensor("bilinear_w", [E, E], f32, kind="ExternalInput").ap()
    q_d = nc.dram_tensor("query", [N, E], f32, kind="ExternalInput").ap()
    v_d = nc.dram_tensor("value", [N, F], f32, kind="ExternalInput").ap()
    o_d = nc.dram_tensor("out", [B_loc, N, E], f32, kind="ExternalOutput").ap()

    K0 = 0.5 * CBAR + 1.0 / (128.0 * CBAR)
    KW = 1.0 / (128.0 * CBAR)

    with tile.TileContext(nc) as tc:
        with (
            tc.tile_pool(name="const", bufs=1) as constp,
            tc.tile_pool(name="xbf", bufs=4) as xbfp,
            tc.tile_pool(name="xtg", bufs=4) as xtgp,
            tc.tile_pool(name="xs", bufs=4) as xsp,
            tc.tile_pool(name="mb", bufs=2) as mbp,
            tc.tile_pool(name="aw", bufs=2) as awp,
            tc.tile_pool(name="st", bufs=2) as stp,
            tc.tile_pool(name="sm", bufs=2) as smp,
            tc.tile_pool(name="awt", bufs=2) as awtp,
            tc.tile_pool(name="osb", bufs=3) as osbp,
            tc.tile_pool(name="pstx", bufs=2, space="PSUM") as pstxp,
            tc.tile_pool(name="ps12", bufs=2, space="PSUM") as ps12p,
            tc.tile_pool(name="ps3", bufs=3, space="PSUM") as ps3p,
            tc.tile_pool(name="psaw", bufs=1, space="PSUM") as psawp,
        ):
            # ---------------- constants ----------------
            ident = constp.tile([128, 128], f32)
            masks.make_identity(nc, ident[:])
            ident_bf = constp.tile([128, 128], bf16, tag="identbf")
            nc.gpsimd.tensor_copy(ident_bf[:], ident[:])

            v2 = constp.tile([128, F], f32)
            nc.sync.dma_start(v2[0:64, :], v_d[:, :])
            nc.sync.dma_start(v2[64:128, :], v_d[:, :])

            wt = {}
            for di in range(2):
                for ej in range(2):
                    t = constp.tile([128, 128], f32, tag=f"wt{di}{ej}")
                    nc.sync.dma_start(
                        t[:], w_d[di * 128:(di + 1) * 128, ej * 128:(ej + 1) * 128]
                    )
                    wt[di, ej] = t

            qtin = []
            for di in range(2):
                t = constp.tile([128, N], f32, tag=f"qtin{di}")
                nc.sync.dma_start(
                    t[:], q_d[:, di * 128:(di + 1) * 128].transpose([1, 0])
                )
                qtin.append(t)

            # qtilde^T = W.T-contract: qt[e, n] = sum_d W[d, e] q[n, d], then * SCALE
            qt_bf = []
            for ej in range(2):
                ps = ps12p.tile([128, 512], f32, tag="ps12")
                for di in range(2):
                    nc.tensor.matmul(
                        ps[:, 0:N], wt[di, ej][:], qtin[di][:],
                        start=(di == 0), stop=(di == 1),
                    )
                t = constp.tile([128, N], bf16, tag=f"qtbf{ej}")
                nc.scalar.mul(t[:], ps[:, 0:N], SCALE)
                qt_bf.append(t)

            # ---------------- per-group emit helpers ----------------
            def emit_front(g):
                """Loads + PE transposes + stage-12 for group g.
                Returns (xbf, xs_t)."""
                b0 = g * C
                xbf = xbfp.tile([128, C, E], bf16, tag="xbf")
                for q in range(C // 8):
                    s0 = q * 8
                    nc.gpsimd.dma_start(
                        xbf[0:64, s0:s0 + 8, :],
                        x_d[b0 + s0:b0 + s0 + 8, :, :].transpose([1, 0, 2]),
                    )
                    nc.gpsimd.dma_start(
                        xbf[64:128, s0:s0 + 8, :],
                        x_d[HALF + b0 + s0:HALF + b0 + s0 + 8, :, :]
                        .transpose([1, 0, 2]),
                    )

                xtg = xtgp.tile([128, C, 2, 128], bf16, tag="xtg")
                for c4 in range(C // 4):
                    pst = pstxp.tile([128, 4, 2, 128], bf16, tag="pstx")
                    for k in range(4):
                        c = c4 * 4 + k
                        for ec in range(2):
                            nc.tensor.transpose(
                                pst[:, k, ec, :],
                                xbf[:, c, ec * 128:(ec + 1) * 128],
                                ident_bf[:],
                            )
                    nc.scalar.copy(xtg[:, c4 * 4:c4 * 4 + 4, :, :], pst[:])

                xs_t = xsp.tile([128, C, 72], f32, tag="xs")
                xs3 = xs_t[:, :, 0:F]
                for blk in range(C // 8):
                    ps = ps12p.tile([128, 512], f32, tag="ps12")
                    for ec in range(2):
                        for s in range(8):
                            c = blk * 8 + s
                            # start=True clears has_written for the MM's
                            # partition rows across the WHOLE bank — only
                            # the first MM of each row-half may set it.
                            nc.tensor.matmul(
                                ps[0:64, s * 64:(s + 1) * 64],
                                qt_bf[ec][:],
                                xtg[:, c, ec, 0:64],
                                start=(ec == 0 and s == 0), stop=(ec == 1),
                                tile_position=(0, 0),
                                skip_group_check=True,
                            )
                            nc.tensor.matmul(
                                ps[64:128, s * 64:(s + 1) * 64],
                                qt_bf[ec][:],
                                xtg[:, c, ec, 64:128],
                                start=(ec == 0 and s == 0), stop=(ec == 1),
                                tile_position=(0, 64),
                                skip_group_check=True,
                            )
                    nc.vector.tensor_copy(
                        xs3[:, blk * 8:(blk + 1) * 8, :],
                        ps[:].rearrange("p (c f) -> p c f", f=F),
                    )
                return xbf, xs_t

            def emit_entmax(xs_t):
                """Generator: entmax Newton solve; yields after each
                instruction so two groups can interleave.  Yields the
                (mb_t, recs2) result tiles via StopIteration value."""
                xs3 = xs_t[:, :, 0:F]
                st = stp.tile([128, C, 8], f32, tag="st")

                def sl(k):
                    return st[:, :, k:k + 1]        # [128, C, 1]

                tau = smp.tile([128, C], f32, tag="tau")
                tauu = tau[:].unsqueeze(2)          # [128, C, 1]

                # init moments
                for c in range(C):
                    nc.vector.bn_stats(st[:, c, 0:6], xs3[:, c, :])
                    yield
                msum = smp.tile([128, C], f32, tag="msum")
                wsum = smp.tile([128, C], f32, tag="wsum")
                nc.vector.tensor_add(msum[:].unsqueeze(2), sl(1), sl(4))
                yield
                nc.vector.tensor_add(wsum[:].unsqueeze(2), sl(2), sl(5))
                yield
                nc.vector.tensor_scalar(
                    out=msum[:], in0=msum[:], scalar1=0.5, scalar2=K0,
                    op0=Alu.mult, op1=Alu.subtract,
                )
                yield
                nc.vector.scalar_tensor_tensor(
                    out=tau[:], in0=wsum[:], scalar=KW, in1=msum[:],
                    op0=Alu.mult, op1=Alu.add,
                )
                yield

                mb_t = mbp.tile([128, C, 72], f32, tag="mb")
                mb3 = mb_t[:, :, 0:F]
                taub = tauu.broadcast_to([128, C, F])

                a2 = smp.tile([128, C, 2], f32, tag="a2")
                u2 = smp.tile([128, C, 2], f32, tag="u2")
                s1m = smp.tile([128, C], f32, tag="s1m")
                s2s = smp.tile([128, C], f32, tag="s2s")
                rcp = smp.tile([128, C], f32, tag="rcp")

                for it in range(NEWTON_ITERS + 1):
                    nc.vector.tensor_max(mb3, xs3, taub)
                    yield
                    for c in range(C):
                        nc.vector.bn_stats(st[:, c, 0:6], mb3[:, c, :])
                        yield
                    nc.vector.tensor_sub(a2[:, :, 0:1], sl(1), tauu)
                    yield
                    nc.vector.tensor_sub(a2[:, :, 1:2], sl(4), tauu)
                    yield
                    nc.scalar.square(u2[:], a2[:])
                    yield
                    nc.vector.scalar_tensor_tensor(
                        out=u2[:, :, 0:1], in0=u2[:, :, 0:1], scalar=32.0,
                        in1=sl(2), op0=Alu.mult, op1=Alu.add,
                    )
                    yield
                    nc.vector.scalar_tensor_tensor(
                        out=u2[:, :, 1:2], in0=u2[:, :, 1:2], scalar=32.0,
                        in1=sl(5), op0=Alu.mult, op1=Alu.add,
                    )
                    yield
                    nc.vector.tensor_reduce(
                        s2s[:], u2[:], axis=mybir.AxisListType.X, op=Alu.add,
                    )
                    yield
                    if it < NEWTON_ITERS:
                        nc.vector.tensor_reduce(
                            s1m[:], a2[:], axis=mybir.AxisListType.X, op=Alu.add,
                        )
                        yield
                        nc.vector.reciprocal(rcp[:], s1m[:])
                        yield
                        nc.vector.tensor_scalar(
                            out=s2s[:], in0=s2s[:], scalar1=-1.0, scalar2=None,
                            op0=Alu.add,
                        )
                        yield
                        nc.vector.tensor_mul(s2s[:], s2s[:], rcp[:])
                        yield
                        nc.vector.scalar_tensor_tensor(
                            out=tau[:], in0=s2s[:], scalar=1.0 / 64.0, in1=tau[:],
                            op0=Alu.mult, op1=Alu.add,
                        )
                        yield

                # final: recip_s2, d = m - tau
                recs2 = smp.tile([128, C], f32, tag="recs2")
                nc.vector.reciprocal(recs2[:], s2s[:])
                yield
                nc.vector.tensor_sub(mb3, mb3, taub)
                yield
                return mb_t, recs2

            def emit_back(g, xbf, mb_t, recs2):
                """aw construction + stage-3 + stores for group g."""
                b0 = g * C
                mb3 = mb_t[:, :, 0:F]
                aw_t = awp.tile([128, C * F], f32, tag="aw")
                aw3 = aw_t[:].rearrange("p (c f) -> p c f", f=F)
                nc.scalar.square(aw3, mb3)
                v2b = v2[:].unsqueeze(1).broadcast_to([128, C, F])
                nc.gpsimd.tensor_mul(aw3, aw3, v2b)
                recs2b = recs2[:].unsqueeze(2).broadcast_to([128, C, F])
                nc.vector.tensor_mul(aw3, aw3, recs2b)

                # aw^T: PE transposes + cast evac; one shift DMA/group
                awt_g = awtp.tile([128, C, 64], bf16, tag="awt")
                awt_tmp = awtp.tile([64, C, 64], bf16, tag="awt_tmp")
                for blk in range(C // 4):
                    pst = psawp.tile([64, 512], f32, tag="psaw")
                    for s in range(4):
                        c = blk * 4 + s
                        nc.tensor.transpose(
                            pst[0:64, s * 128:(s + 1) * 128],
                            aw_t[:, c * 64:(c + 1) * 64],
                            ident[:],
                        )
                    pst3 = pst[:].rearrange("p (s h f) -> p s h f", s=4, h=2)
                    nc.scalar.copy(
                        awt_g[0:64, blk * 4:(blk + 1) * 4, :], pst3[:, :, 0, :]
                    )
                    nc.scalar.copy(
                        awt_tmp[:, blk * 4:(blk + 1) * 4, :], pst3[:, :, 1, :]
                    )
                nc.sync.dma_start(awt_g[64:128, :, :], awt_tmp[:])

                # stage-3 (bf16): out = exp(awt.T @ x), staged per half-group
                for hb in range(2):
                    osb = osbp.tile([128, C // 2, E], f32, tag="osb")
                    for p2 in range(C // 4):
                        ps3 = ps3p.tile([128, 512], f32, tag="ps3")
                        for sl2 in range(2):
                            c = hb * (C // 2) + p2 * 2 + sl2
                            nc.tensor.matmul(
                                ps3[0:64, sl2 * 256:(sl2 + 1) * 256],
                                awt_g[0:64, c, :],
                                xbf[0:64, c, :],
                                start=True, stop=True,
                                tile_position=(0, 0),
                                skip_group_check=True,
                            )
                            nc.tensor.matmul(
                                ps3[64:128, sl2 * 256:(sl2 + 1) * 256],
                                awt_g[64:128, c, :],
                                xbf[64:128, c, :],
                                start=True, stop=True,
                                tile_position=(64, 64),
                                skip_group_check=True,
                            )
                        nc.scalar.activation(
                            osb[:, p2 * 2:p2 * 2 + 2, :], ps3[:],
                            ACTF.Exp,
                        )
                    bA = b0 + hb * (C // 2)
                    nc.sync.dma_start(
                        o_d[bA:bA + C // 2, :, :].transpose([1, 0, 2]),
                        osb[0:64, :, :],
                    )
                    nc.sync.dma_start(
                        o_d[HALF + bA:HALF + bA + C // 2, :, :]
                        .transpose([1, 0, 2]),
                        osb[64:128, :, :],
                    )

            # ---------------- paired-group supersteps ----------------
            for gp in range(NG // 2):
                gA, gB = 2 * gp, 2 * gp + 1
                xbfA, xsA = emit_front(gA)
                xbfB, xsB = emit_front(gB)
                # interleave the two entmax instruction streams
                genA, genB = emit_entmax(xsA), emit_entmax(xsB)
                resA = resB = None
                while resA is None or resB is None:
                    if resA is None:
                        try:
                            next(genA)
                        except StopIteration as e:
                            resA = e.value
                    if resB is None:
                        try:
                            next(genB)
                        except StopIteration as e:
                            resB = e.value
                emit_back(gA, xbfA, *resA)
                emit_back(gB, xbfB, *resB)
    if not nc.is_finalized():
        nc.finalize()
    return nc


_NC_CACHE = {}


def _get_program(B_loc, NG):
    key = (B_loc, NG)
    if key not in _NC_CACHE:
        _NC_CACHE[key] = build_program(B_loc, NG)
    return _NC_CACHE[key]


def kernel(**inputs):
    from concourse.bass_utils import run_bass_kernel_spmd

    x = np.ascontiguousarray(np.asarray(inputs["x"], dtype=np.float32))
    w = np.ascontiguousarray(np.asarray(inputs["bilinear_w"], dtype=np.float32))
    q = np.ascontiguousarray(np.asarray(inputs["query"], dtype=np.float32))
    v = np.ascontiguousarray(np.asarray(inputs["value"], dtype=np.float32))
    B = x.shape[0]
    B_loc = B // NCORES

    nc = _get_program(B_loc, 8)

    in_maps = []
    for core in range(NCORES):
        sh = x[core * B_loc:(core + 1) * B_loc]
        in_maps.append(
            {"x": np.ascontiguousarray(sh), "bilinear_w": w, "query": q, "value": v}
        )

    import os
    trace = bool(int(os.environ.get("KERNEL_TRACE", "0")))
    res = run_bass_kernel_spmd(
        nc, in_maps, core_ids=list(range(NCORES)), trace=trace,
        trace_cores=[0] if trace else None,
    )
    if trace:
        kernel.last_exec_time_ns = res.exec_time_ns
        kernel.last_trace = res.instructions_and_trace
    out = np.concatenate([r["out"] for r in res.results], axis=0)
    return out


if __name__ == "__main__":
    # smoke-test the builder only
    nc = build_program(32, 2)
    print("build ok:", len(nc.inst_map), "instructions")
